# revision 45
# baseline (speedup 1.0000x reference)
"""Grouped per-channel Linear + ReLU on 8 TRN2 NeuronCores.

Problem: out[b,c,e] = relu(sum_s x[b,s,c] * W[c,s,e] + bias[c,e])
  x: (256, 2048, 32) f32, W: (32, 2048, 2048) f32, bias: (32, 2048) f32
  out: (256, 32, 2048) f32

Sharding: expert/channel parallel — core i computes channels [4i, 4i+4).
Each core runs 4 independent GEMMs of (256x2048)@(2048x2048) with the
contraction dim S on SBUF partitions. x is pre-transposed on the host to
(C, S, B) so both matmul operands stream from DRAM with contiguous rows.
Everything is cast to fp16 on the host: the values are O(1), so fp16's
extra mantissa bits give ~3.6e-4 rel error (vs ~2.4e-3 for bf16) at the
same full-rate matmul speed and half the fp32 HBM traffic.

Per channel: W streams on the SP-HWDGE (sync) ring in 2 MB chunks
(4 k-tiles x full E row; channel 0 uses a ramp of small chunks on the
otherwise-idle ACT ring so the PE starts ~12 us in). The x slab (1 MB)
prefetches on the SWDGE (gpsimd) ring one channel ahead, gated by a
mid-channel W chunk so it can't steal HBM bandwidth from the live W
stream. All 8 PSUM banks hold the (2 bt x 4 et) output block accumulating
over 16 k-tiles. Eviction: VectorE adds the partition-broadcast bias
(freeing the PSUM bank), ScalarE applies ReLU + fp16 cast, and outputs
leave as 0.5 MB DMAs on the ACT-HWDGE ring (per-subtile eager DMAs on the
last channel to shorten the kernel tail).

Measured on 8 axon-tunneled TRN2 cores: ~143-155 us HW exec (max core),
rel l2 error 3.6e-4. Per-core roofline: PE ~117 us (512 N=512 fp16
matmuls), DMA ~112 us (40 MB at ~358 GB/s HBM cap) — the kernel runs
within ~10% of the joint floor including fixed NEFF preamble/drain.
"""

import os
import sys

for _p in ("/opt/trn_rl_repo", "/root/.axon_site/_ro/trn_rl_repo"):
    if os.path.isdir(_p) and _p not in sys.path:
        sys.path.insert(0, _p)

import numpy as np
import ml_dtypes

import concourse.bacc as bacc
import concourse.mybir as mybir
from concourse import tile
from concourse.bass_utils import run_bass_kernel_spmd
from concourse.tile_rust import add_dep_helper

B, S, C, E = 256, 2048, 32, 2048
NCORES = 8
CPC = C // NCORES          # channels per core = 4
P = 128
KT = S // P                # 16 k-tiles
NBT = B // P               # 2 batch tiles
FREE = 512                 # matmul moving free dim (one PSUM bank of f32)
NET = E // FREE            # 4 e-tiles
KC = 8                     # k-tiles per W DMA chunk (4 MB chunks)

# matmul dtype: "float16" (fast, ~3e-4 rel: values are O(1) so fp16's extra
# mantissa bits beat bf16 at the same matmul speed), "bfloat16" (fast,
# ~2e-3 rel), "float32r" (~1.5e-4 rel, DMA-bound 2x slower), "float32"
# (exact, 4x slower compute-bound)
MM_DTYPE = os.environ.get("KERNEL_MM_DTYPE", "float16")

# (mybir mm dtype, numpy input dtype, mybir out dtype, numpy out dtype)
_DTYPES = {
    "float16": (mybir.dt.float16, np.float16, mybir.dt.float16, np.float16),
    "bfloat16": (
        mybir.dt.bfloat16,
        ml_dtypes.bfloat16,
        mybir.dt.float16,
        np.float16,
    ),
    "float32r": (mybir.dt.float32r, np.float32, mybir.dt.float32, np.float32),
    "float32": (mybir.dt.float32, np.float32, mybir.dt.float32, np.float32),
}

_nc_cache = {}


def _build(mm_dtype: str):
    mm_dt, _, out_dt, _ = _DTYPES[mm_dtype]
    nc = bacc.Bacc(None, target_bir_lowering=False)
    xt = nc.dram_tensor("xt", [CPC, S, B], mm_dt, kind="ExternalInput")
    w = nc.dram_tensor("w", [CPC, S, E], mm_dt, kind="ExternalInput")
    bias = nc.dram_tensor("bias", [CPC, E], mybir.dt.float32, kind="ExternalInput")
    out = nc.dram_tensor("out", [B, CPC, E], out_dt, kind="ExternalOutput")

    with tile.TileContext(nc) as tc:
        with (
            tc.tile_pool(name="const", bufs=1) as const,
            tc.tile_pool(name="xpool", bufs=2) as xpool,
            tc.tile_pool(name="bpool", bufs=2) as bpool,
            tc.tile_pool(name="bbpool", bufs=2) as bbpool,
            tc.tile_pool(name="ttmp", bufs=4) as ttmp,
            tc.tile_pool(name="wpool", bufs=3) as wpool,
            tc.tile_pool(name="opool", bufs=3) as opool,
            tc.tile_pool(name="psum", bufs=NBT * NET, space="PSUM") as psum,
        ):
            zbias = const.tile([P, 1], mybir.dt.float32)
            nc.any.memset(zbias[:], 0.0)

            # x slab + bias per channel, created lazily so prefetches can be
            # emitted from inside the previous channel's compute loop.
            xtiles: dict[int, object] = {}
            btiles: dict[int, object] = {}

            def prefetch_channel(c, eng, split=False, after=None):
                xsb = xpool.tile([P, KT, B], mm_dt, name="xsb")
                if split:
                    # two 512 KB pieces so the first matmuls start sooner
                    h = KT // 2
                    eng.dma_start(
                        xsb[:, :h, :],
                        xt[c, : h * P, :].rearrange("(k p) b -> p k b", p=P),
                    )
                    eng.dma_start(
                        xsb[:, h:, :],
                        xt[c, h * P :, :].rearrange("(k p) b -> p k b", p=P),
                    )
                else:
                    xdma = eng.dma_start(
                        xsb[:], xt[c, :, :].rearrange("(k p) b -> p k b", p=P)
                    )
                    if after is not None:
                        # hold the prefetch back until mid-channel so it
                        # doesn't steal HBM bandwidth from the live W stream
                        # (GpSimd is in-order, so this gates the whole batch)
                        add_dep_helper(
                            xdma.ins,
                            after.ins,
                            reason="x prefetch waits for mid-channel W chunk",
                        )
                xtiles[c] = xsb
                bsb = bpool.tile([1, E], mybir.dt.float32, name="bsb")
                nc.gpsimd.dma_start(bsb[:], bias[c : c + 1, :])
                # broadcast the bias row to all 128 partitions once per
                # channel (GpSimd is idle apart from the x DMAs) so eviction
                # can add it element-wise
                bbc = bbpool.tile([P, E], mybir.dt.float32, name="bbc")
                nc.gpsimd.partition_broadcast(bbc[:], bsb[:])
                btiles[c] = bbc

            for c in range(CPC):
                if c == 0:
                    # latency-critical first channel: x pieces ride the fast
                    # SP-HWDGE (sync) ring while the W ramp chunks stream
                    # concurrently on the ACT-HWDGE (scalar) ring
                    prefetch_channel(0, nc.sync, split=True)
                xsb = xtiles[c]

                ps = [
                    [
                        psum.tile([P, FREE], mybir.dt.float32, name="ps")
                        for _ in range(NET)
                    ]
                    for _ in range(NBT)
                ]
                # W chunk schedule (k-tiles per DMA): ramp up at kernel start
                # so the first matmuls don't wait on a full 2 MB transfer.
                chunk_kts = [1, 1, 2, 2, 2, 4, 4] if c == 0 else [KC] * (KT // KC)
                k = 0
                prefetched = False
                for ci, ckt in enumerate(chunk_kts):
                    wsb = wpool.tile([P, KC, E], mm_dt, name="wsb")
                    # channel 0's ramp chunks (k<4) ride the otherwise-idle
                    # ACT ring so they transfer concurrently with the x pieces
                    # on sync and the PE never starves during the ramp
                    weng = nc.scalar if c == 0 and k < 4 else nc.sync
                    wdma = weng.dma_start(
                        wsb[:, :ckt, :],
                        w[c, k * P : (k + ckt) * P, :].rearrange(
                            "(k p) e -> p k e", p=P
                        ),
                    )
                    for kk in range(ckt):
                        for bt in range(NBT):
                            lhsT = xsb[:, k, bt * P : (bt + 1) * P]
                            for et in range(NET):
                                nc.tensor.matmul(
                                    ps[bt][et][:],
                                    lhsT,
                                    wsb[:, kk, et * FREE : (et + 1) * FREE],
                                    start=(k == 0),
                                    stop=(k == KT - 1),
                                )
                        k += 1
                    if not prefetched and k >= KC and c + 1 < CPC:
                        # prefetch next channel's x + bias on the SWDGE ring
                        # while this channel still has half its compute left
                        prefetch_channel(c + 1, nc.gpsimd, after=wdma)
                        prefetched = True
                # Evict: VectorE adds the broadcast bias (freeing the PSUM
                # bank), ScalarE applies ReLU + fp16 cast.
                bbc = btiles[c]
                last = c == CPC - 1
                for bt in range(NBT):
                    ot = opool.tile([P, E], out_dt)
                    for et in range(NET):
                        dst = ot[:, et * FREE : (et + 1) * FREE]
                        tmp = ttmp.tile([P, FREE], mybir.dt.float32, name="tmp")
                        nc.vector.tensor_add(
                            tmp[:],
                            ps[bt][et][:],
                            bbc[:, et * FREE : (et + 1) * FREE],
                        )
                        nc.scalar.activation(
                            dst,
                            tmp[:],
                            mybir.ActivationFunctionType.Relu,
                            bias=zbias[:],
                        )
                        if last:
                            # tail: small eager DMAs, alternating across both
                            # HWDGE rings (the W stream is finished by now)
                            oeng = nc.sync if et % 2 == 0 else nc.scalar
                            oeng.dma_start(
                                out[
                                    bt * P : (bt + 1) * P,
                                    c,
                                    et * FREE : (et + 1) * FREE,
                                ],
                                dst,
                            )
                    if not last:
                        # one 1 MB DMA per (bt, c) on the ACT HWDGE ring,
                        # separate from the W stream
                        nc.scalar.dma_start(out[bt * P : (bt + 1) * P, c, :], ot[:])
    nc.compile()
    return nc


def _get_nc(mm_dtype: str):
    if mm_dtype not in _nc_cache:
        _nc_cache[mm_dtype] = _build(mm_dtype)
    return _nc_cache[mm_dtype]


def _run(x, W, b, mm_dtype=None, **spmd_kwargs):
    mm_dtype = mm_dtype or MM_DTYPE
    _, np_dt, _, _ = _DTYPES[mm_dtype]
    nc = _get_nc(mm_dtype)

    in_maps = []
    for i in range(NCORES):
        c0, c1 = i * CPC, (i + 1) * CPC
        xt_i = np.ascontiguousarray(
            x[:, :, c0:c1].transpose(2, 1, 0).astype(np_dt)
        )
        w_i = np.ascontiguousarray(W[c0:c1].astype(np_dt))
        b_i = np.ascontiguousarray(b[c0:c1].astype(np.float32))
        in_maps.append({"xt": xt_i, "w": w_i, "bias": b_i})

    res = run_bass_kernel_spmd(nc, in_maps, core_ids=list(range(NCORES)), **spmd_kwargs)
    out = np.concatenate(
        [r["out"].astype(np.float32) for r in res.results], axis=1
    )
    return out, res


def kernel(x: np.ndarray, W: np.ndarray, b: np.ndarray) -> np.ndarray:
    out, _ = _run(x, W, b)
    return out


# revision 47
# speedup vs baseline: 1.0721x; 1.0721x over previous
"""Grouped per-channel Linear + ReLU on 8 TRN2 NeuronCores.

Problem: out[b,c,e] = relu(sum_s x[b,s,c] * W[c,s,e] + bias[c,e])
  x: (256, 2048, 32) f32, W: (32, 2048, 2048) f32, bias: (32, 2048) f32
  out: (256, 32, 2048) f32

Sharding: expert/channel parallel — core i computes channels [4i, 4i+4).
Each core runs 4 independent GEMMs of (256x2048)@(2048x2048) with the
contraction dim S on SBUF partitions. x is pre-transposed on the host to
(C, S, B) so both matmul operands stream from DRAM with contiguous rows.
Everything is cast to fp16 on the host: the values are O(1), so fp16's
extra mantissa bits give ~3.6e-4 rel error (vs ~2.4e-3 for bf16) at the
same full-rate matmul speed and half the fp32 HBM traffic.

Per channel: W streams on the SP-HWDGE (sync) ring in 2 MB chunks
(4 k-tiles x full E row; channel 0 uses a ramp of small chunks on the
otherwise-idle ACT ring so the PE starts ~12 us in). The x slab (1 MB)
prefetches on the SWDGE (gpsimd) ring one channel ahead, gated by a
mid-channel W chunk so it can't steal HBM bandwidth from the live W
stream. All 8 PSUM banks hold the (2 bt x 4 et) output block accumulating
over 16 k-tiles. Eviction: VectorE adds the partition-broadcast bias
(freeing the PSUM bank), ScalarE applies ReLU + fp16 cast, and outputs
leave as 0.5 MB DMAs on the ACT-HWDGE ring (per-subtile eager DMAs on the
last channel to shorten the kernel tail).

Measured on 8 axon-tunneled TRN2 cores: ~143-155 us HW exec (max core),
rel l2 error 3.6e-4. Per-core roofline: PE ~117 us (512 N=512 fp16
matmuls), DMA ~112 us (40 MB at ~358 GB/s HBM cap) — the kernel runs
within ~10% of the joint floor including fixed NEFF preamble/drain.
"""

import os
import sys

for _p in ("/opt/trn_rl_repo", "/root/.axon_site/_ro/trn_rl_repo"):
    if os.path.isdir(_p) and _p not in sys.path:
        sys.path.insert(0, _p)

import numpy as np
import ml_dtypes

import concourse.bacc as bacc
import concourse.mybir as mybir
from concourse import tile
from concourse.bass_utils import run_bass_kernel_spmd
from concourse.tile_rust import add_dep_helper

B, S, C, E = 256, 2048, 32, 2048
NCORES = 8
CPC = C // NCORES          # channels per core = 4
P = 128
KT = S // P                # 16 k-tiles
NBT = B // P               # 2 batch tiles
FREE = 512                 # matmul moving free dim (one PSUM bank of f32)
NET = E // FREE            # 4 e-tiles
KC = 4                     # k-tiles per W DMA chunk (2 MB chunks)

# matmul dtype: "float16" (fast, ~3e-4 rel: values are O(1) so fp16's extra
# mantissa bits beat bf16 at the same matmul speed), "bfloat16" (fast,
# ~2e-3 rel), "float32r" (~1.5e-4 rel, DMA-bound 2x slower), "float32"
# (exact, 4x slower compute-bound)
MM_DTYPE = os.environ.get("KERNEL_MM_DTYPE", "float16")

# (mybir mm dtype, numpy input dtype, mybir out dtype, numpy out dtype)
_DTYPES = {
    "float16": (mybir.dt.float16, np.float16, mybir.dt.float16, np.float16),
    "bfloat16": (
        mybir.dt.bfloat16,
        ml_dtypes.bfloat16,
        mybir.dt.float16,
        np.float16,
    ),
    "float32r": (mybir.dt.float32r, np.float32, mybir.dt.float32, np.float32),
    "float32": (mybir.dt.float32, np.float32, mybir.dt.float32, np.float32),
}

_nc_cache = {}


def _build(mm_dtype: str):
    mm_dt, _, out_dt, _ = _DTYPES[mm_dtype]
    nc = bacc.Bacc(None, target_bir_lowering=False)
    xt = nc.dram_tensor("xt", [CPC, S, B], mm_dt, kind="ExternalInput")
    w = nc.dram_tensor("w", [CPC, S, E], mm_dt, kind="ExternalInput")
    bias = nc.dram_tensor("bias", [CPC, E], mybir.dt.float32, kind="ExternalInput")
    out = nc.dram_tensor("out", [B, CPC, E], out_dt, kind="ExternalOutput")

    with tile.TileContext(nc) as tc:
        with (
            tc.tile_pool(name="const", bufs=1) as const,
            tc.tile_pool(name="xpool", bufs=2) as xpool,
            tc.tile_pool(name="bpool", bufs=2) as bpool,
            tc.tile_pool(name="bbpool", bufs=2) as bbpool,
            tc.tile_pool(name="ttmp", bufs=4) as ttmp,
            tc.tile_pool(name="wpool", bufs=5) as wpool,
            tc.tile_pool(name="opool", bufs=3) as opool,
            tc.tile_pool(name="psum", bufs=NBT * NET, space="PSUM") as psum,
        ):
            zbias = const.tile([P, 1], mybir.dt.float32)
            nc.any.memset(zbias[:], 0.0)

            # x slab + bias per channel, created lazily so prefetches can be
            # emitted from inside the previous channel's compute loop.
            xtiles: dict[int, object] = {}
            btiles: dict[int, object] = {}

            def prefetch_channel(c, eng, split=False, after=None):
                xsb = xpool.tile([P, KT, B], mm_dt, name="xsb")
                if split:
                    # two 512 KB pieces so the first matmuls start sooner
                    h = KT // 2
                    eng.dma_start(
                        xsb[:, :h, :],
                        xt[c, : h * P, :].rearrange("(k p) b -> p k b", p=P),
                    )
                    eng.dma_start(
                        xsb[:, h:, :],
                        xt[c, h * P :, :].rearrange("(k p) b -> p k b", p=P),
                    )
                else:
                    xdma = eng.dma_start(
                        xsb[:], xt[c, :, :].rearrange("(k p) b -> p k b", p=P)
                    )
                    if after is not None:
                        # hold the prefetch back until mid-channel so it
                        # doesn't steal HBM bandwidth from the live W stream
                        # (GpSimd is in-order, so this gates the whole batch)
                        add_dep_helper(
                            xdma.ins,
                            after.ins,
                            reason="x prefetch waits for mid-channel W chunk",
                        )
                xtiles[c] = xsb
                bsb = bpool.tile([1, E], mybir.dt.float32, name="bsb")
                nc.gpsimd.dma_start(bsb[:], bias[c : c + 1, :])
                # broadcast the bias row to all 128 partitions once per
                # channel (GpSimd is idle apart from the x DMAs) so eviction
                # can add it element-wise
                bbc = bbpool.tile([P, E], mybir.dt.float32, name="bbc")
                nc.gpsimd.partition_broadcast(bbc[:], bsb[:])
                btiles[c] = bbc

            for c in range(CPC):
                if c == 0:
                    # latency-critical first channel: x pieces ride the fast
                    # SP-HWDGE (sync) ring while the W ramp chunks stream
                    # concurrently on the ACT-HWDGE (scalar) ring
                    prefetch_channel(0, nc.sync, split=True)
                xsb = xtiles[c]

                ps = [
                    [
                        psum.tile([P, FREE], mybir.dt.float32, name="ps")
                        for _ in range(NET)
                    ]
                    for _ in range(NBT)
                ]
                # W chunk schedule (k-tiles per DMA): ramp up at kernel start
                # so the first matmuls don't wait on a full 2 MB transfer.
                chunk_kts = [1, 1, 2, 2, 2, 4, 4] if c == 0 else [KC] * (KT // KC)
                k = 0
                prefetched = False
                for ci, ckt in enumerate(chunk_kts):
                    wsb = wpool.tile([P, KC, E], mm_dt, name="wsb")
                    # channel 0's ramp chunks (k<4) ride the otherwise-idle
                    # ACT ring so they transfer concurrently with the x pieces
                    # on sync and the PE never starves during the ramp
                    weng = nc.scalar if c == 0 and k < 4 else nc.sync
                    wdma = weng.dma_start(
                        wsb[:, :ckt, :],
                        w[c, k * P : (k + ckt) * P, :].rearrange(
                            "(k p) e -> p k e", p=P
                        ),
                    )
                    if c > 0 and ci == 0:
                        # First chunk of a new channel: bank-major order (all
                        # KC k-tiles per PSUM bank before the next bank).
                        # The previous channel's banks free at the DVE-add
                        # eviction rate (~0.68us each); bank-major gives the
                        # in-order PE ~0.9us of work per freed bank instead
                        # of ~0.22us, hiding the eviction latency.
                        for bt in range(NBT):
                            for et in range(NET):
                                for kk in range(ckt):
                                    nc.tensor.matmul(
                                        ps[bt][et][:],
                                        xsb[:, kk, bt * P : (bt + 1) * P],
                                        wsb[:, kk, et * FREE : (et + 1) * FREE],
                                        start=(kk == 0),
                                        stop=False,
                                    )
                        k += ckt
                    else:
                        for kk in range(ckt):
                            for bt in range(NBT):
                                lhsT = xsb[:, k, bt * P : (bt + 1) * P]
                                for et in range(NET):
                                    nc.tensor.matmul(
                                        ps[bt][et][:],
                                        lhsT,
                                        wsb[:, kk, et * FREE : (et + 1) * FREE],
                                        start=(k == 0),
                                        stop=(k == KT - 1),
                                    )
                            k += 1
                    if not prefetched and k >= KC and c + 1 < CPC:
                        # prefetch next channel's x + bias on the SWDGE ring
                        # while this channel still has half its compute left
                        prefetch_channel(c + 1, nc.gpsimd, after=wdma)
                        prefetched = True
                # Evict: VectorE adds the broadcast bias (freeing the PSUM
                # bank), ScalarE applies ReLU + fp16 cast.
                bbc = btiles[c]
                last = c == CPC - 1
                for bt in range(NBT):
                    ot = opool.tile([P, E], out_dt)
                    for et in range(NET):
                        dst = ot[:, et * FREE : (et + 1) * FREE]
                        tmp = ttmp.tile([P, FREE], mybir.dt.float32, name="tmp")
                        nc.vector.tensor_add(
                            tmp[:],
                            ps[bt][et][:],
                            bbc[:, et * FREE : (et + 1) * FREE],
                        )
                        nc.scalar.activation(
                            dst,
                            tmp[:],
                            mybir.ActivationFunctionType.Relu,
                            bias=zbias[:],
                        )
                        if last:
                            # tail: small eager DMAs, alternating across both
                            # HWDGE rings (the W stream is finished by now)
                            oeng = nc.sync if et % 2 == 0 else nc.scalar
                            oeng.dma_start(
                                out[
                                    bt * P : (bt + 1) * P,
                                    c,
                                    et * FREE : (et + 1) * FREE,
                                ],
                                dst,
                            )
                    if not last:
                        # one 1 MB DMA per (bt, c) on the ACT HWDGE ring,
                        # separate from the W stream
                        nc.scalar.dma_start(out[bt * P : (bt + 1) * P, c, :], ot[:])
    nc.compile()
    return nc


def _get_nc(mm_dtype: str):
    if mm_dtype not in _nc_cache:
        _nc_cache[mm_dtype] = _build(mm_dtype)
    return _nc_cache[mm_dtype]


def _run(x, W, b, mm_dtype=None, **spmd_kwargs):
    mm_dtype = mm_dtype or MM_DTYPE
    _, np_dt, _, _ = _DTYPES[mm_dtype]
    nc = _get_nc(mm_dtype)

    in_maps = []
    for i in range(NCORES):
        c0, c1 = i * CPC, (i + 1) * CPC
        xt_i = np.ascontiguousarray(
            x[:, :, c0:c1].transpose(2, 1, 0).astype(np_dt)
        )
        w_i = np.ascontiguousarray(W[c0:c1].astype(np_dt))
        b_i = np.ascontiguousarray(b[c0:c1].astype(np.float32))
        in_maps.append({"xt": xt_i, "w": w_i, "bias": b_i})

    res = run_bass_kernel_spmd(nc, in_maps, core_ids=list(range(NCORES)), **spmd_kwargs)
    out = np.concatenate(
        [r["out"].astype(np.float32) for r in res.results], axis=1
    )
    return out, res


def kernel(x: np.ndarray, W: np.ndarray, b: np.ndarray) -> np.ndarray:
    out, _ = _run(x, W, b)
    return out
